# revision 7
# baseline (speedup 1.0000x reference)
"""RNN-T JointNetwork Trainium2 kernel.

logits[b,t,u,v] = sum_j W_out[v,j] * tanh(f[b,t,j] + g[b,u,j]) + b_out[v]
  f = enc_out @ W_enc.T   [B,T,640]
  g = pred_out @ W_pred.T [B,U,640]

Sharding: data-parallel over B=8 across the 8 NeuronCores (1 batch/core).

Per-core device program (everything resident on-chip):
  phase 1: fT = W_enc @ enc.T -> [640,256] f32 accumulated in PSUM (stays
           there; ScalarE reads PSUM faster than SBUF), gT -> [640,64]
           copied to SBUF (activation bias operands must be SBUF).
           Inputs bf16 (host-cast) so phase 1 runs at full PE rate.
  phase 2: per u: combT_u[j,t] = tanh(fT + gT[:,u]) via ScalarE activation
           with per-partition bias (u-major ordering turns the broadcast
           into a partition-axis bias), output cast to bf16
  phase 3: logits rows = combT_u.T @ W_outT in bf16, K=640 as 5x128 chunks
           accumulated into PSUM [128 rows, 512 vocab]
  phase 4: VectorE adds bias -> f32 row tile, then per-(t,u)-row absmax
           int8 quantization: q = round(logits * 127/absmax), shipped
           off-chip with the f32 dequant scale absmax/127 per row.

Host side: the wall-clock cost of this problem is NOT device compute
(~310us) but host<->device transport of the 537MB f32 logits tensor over
a ~50MB/s tunnel. So the kernel ships int8 logits + per-row scales
(135MB) and the host dequantizes. The exec path is hand-rolled (same
bass_exec primitive bass_utils uses) so the jitted executable is built
once and the donated output buffers are created ON DEVICE instead of
uploading 537MB of host zeros every call.
"""

import sys

for _p in ("/opt/trn_rl_repo",):
    if _p not in sys.path:
        sys.path.insert(0, _p)

import numpy as np
import ml_dtypes

B, T, U = 8, 256, 64
D_ENC, D_PRED, D_JOINT, VOCAB = 512, 512, 640, 1024
KE = D_ENC // 128   # 4 contraction chunks for enc/pred matmuls
KJ = D_JOINT // 128  # 5 contraction chunks for the vocab matmul
N_CORES = 8
RT = T // 128       # 2 row tiles of t per u

_compiled = None
_exec = None


def _build():
    import concourse.bacc as bacc
    import concourse.bass as bass
    import concourse.mybir as mybir
    import concourse.tile as tile

    AX = mybir.AxisListType
    ALU = mybir.AluOpType
    f32 = mybir.dt.float32
    bf16 = mybir.dt.bfloat16
    i8 = mybir.dt.int8
    PSUM = bass.MemorySpace.PSUM
    tanh = mybir.ActivationFunctionType.Tanh

    nc = bacc.Bacc(
        "TRN2",
        target_bir_lowering=False,
        debug=False,
        enable_asserts=False,
    )

    enc_d = nc.dram_tensor("enc", [128, KE, T], bf16, kind="ExternalInput")
    pred_d = nc.dram_tensor("pred", [128, KE, U], bf16, kind="ExternalInput")
    wenc_d = nc.dram_tensor("wenc", [128, KE, D_JOINT], bf16, kind="ExternalInput")
    wpred_d = nc.dram_tensor("wpred", [128, KE, D_JOINT], bf16, kind="ExternalInput")
    wout_d = nc.dram_tensor("wout", [128, KJ, VOCAB], bf16, kind="ExternalInput")
    bias_d = nc.dram_tensor("bias", [1, VOCAB], f32, kind="ExternalInput")
    outq_d = nc.dram_tensor("outq", [T, U, VOCAB], i8, kind="ExternalOutput")
    scale_d = nc.dram_tensor("scale", [T, U], f32, kind="ExternalOutput")

    with tile.TileContext(nc) as tc:
        with (
            tc.tile_pool(name="const", bufs=1) as const,
            tc.tile_pool(name="comb", bufs=3) as comb_pool,
            tc.tile_pool(name="outsb", bufs=4) as out_pool,
            tc.tile_pool(name="qsb", bufs=4) as q_pool,
            tc.tile_pool(name="stat", bufs=8) as stat_pool,
            tc.tile_pool(name="psf", bufs=1, space=PSUM) as psf,
        ):
            # Trigger the Tanh ACT table load before any data arrives.
            warm = const.tile([1, 8], f32)
            warm2 = const.tile([1, 8], f32)
            nc.vector.memset(warm[:], 0.0)
            nc.scalar.activation(warm2[:], warm[:], tanh)

            pred_sb = const.tile([128, KE, U], bf16)
            wpred_sb = const.tile([128, KE, D_JOINT], bf16)
            enc_sb = const.tile([128, KE, T], bf16)
            wenc_sb = const.tile([128, KE, D_JOINT], bf16)
            wout_sb = const.tile([128, KJ, VOCAB], bf16)
            bias_row = const.tile([1, VOCAB], f32)
            bias_sb = const.tile([128, VOCAB], f32)
            ones_sb = const.tile([1, 128], f32)
            gT_sb = const.tile([128, KJ, U], f32)
            scales_sb = const.tile([128, RT, U], f32)
            fT_ps = psf.tile([128, KJ, T], f32)  # 5 KiB/partition -> 3 banks

            # PE warmup: dummy matmuls on zeroed data while input DMAs are
            # in flight, so HAM un-throttles before the real matmuls start.
            wz = const.tile([128, 512], bf16)
            nc.vector.memset(wz[:], 0.0)
            nc.vector.memset(ones_sb[:], 1.0)

            # Input DMA triggers spread across the three DMA-capable
            # engines so they issue in parallel.
            nc.sync.dma_start(pred_sb[:], pred_d[:])
            nc.gpsimd.dma_start(wpred_sb[:], wpred_d[:])
            nc.scalar.dma_start(enc_sb[:], enc_d[:])
            nc.sync.dma_start(wenc_sb[:], wenc_d[:])
            nc.gpsimd.dma_start(wout_sb[:], wout_d[:])
            nc.scalar.dma_start(bias_row[:], bias_d[:])

            with tc.tile_pool(name="psw", bufs=1, space=PSUM) as psw:
                pw = psw.tile([128, 512], f32)
                for i in range(10):
                    nc.tensor.matmul(pw[:], wz[:, :128], wz[:], start=True, stop=True)

            # phase 1: j-outer accumulation groups (a group must fully
            # close before another start=True touches its PSUM bank);
            # gT copies interleave under the following fT matmul group.
            with tc.tile_pool(name="psg", bufs=2, space=PSUM) as psg:
                for j in range(KJ):
                    ps = psg.tile([128, U], f32, tag="psg")
                    for k in range(KE):
                        nc.tensor.matmul(
                            ps[:],
                            wpred_sb[:, k, j * 128:(j + 1) * 128],
                            pred_sb[:, k, :],
                            start=(k == 0),
                            stop=(k == KE - 1),
                        )
                    nc.scalar.copy(gT_sb[:, j, :], ps[:])
                    for k in range(KE):
                        nc.tensor.matmul(
                            fT_ps[:, j, :],
                            wenc_sb[:, k, j * 128:(j + 1) * 128],
                            enc_sb[:, k, :],
                            start=(k == 0),
                            stop=(k == KE - 1),
                        )

                # replicate b_out across partitions with two rank-1 matmuls
                bps = psg.tile([128, 512], f32, tag="psg", name="bps")
                nc.tensor.matmul(bps[:], ones_sb[:], bias_row[:, 0:512],
                                 start=True, stop=True)
                nc.vector.tensor_copy(bias_sb[:, 0:512], bps[:])
                bps2 = psg.tile([128, 512], f32, tag="psg", name="bps2")
                nc.tensor.matmul(bps2[:], ones_sb[:], bias_row[:, 512:1024],
                                 start=True, stop=True)
                nc.vector.tensor_copy(bias_sb[:, 512:1024], bps2[:])

            with tc.tile_pool(name="pso", bufs=5, space=PSUM) as pso:
                for u in range(U):
                    comb = comb_pool.tile([128, KJ, T], bf16, tag="comb")
                    for j in range(KJ):
                        nc.scalar.activation(
                            comb[:, j, :],
                            fT_ps[:, j, :],
                            tanh,
                            bias=gT_sb[:, j, u:u + 1],
                        )
                    for rt in range(RT):
                        rows = slice(rt * 128, (rt + 1) * 128)
                        po0 = pso.tile([128, 512], f32, tag="pso")
                        po1 = pso.tile([128, 512], f32, tag="pso")
                        ob = out_pool.tile([128, VOCAB], f32, tag="ob")
                        qt = q_pool.tile([128, VOCAB], i8, tag="qt")
                        am = stat_pool.tile([128, 1], f32, tag="am")
                        inv = stat_pool.tile([128, 1], f32, tag="inv")
                        for j in range(KJ):
                            lhsT = comb[:, j, rows]
                            nc.tensor.matmul(
                                po0[:], lhsT, wout_sb[:, j, 0:512],
                                start=(j == 0), stop=(j == KJ - 1),
                            )
                            nc.tensor.matmul(
                                po1[:], lhsT, wout_sb[:, j, 512:1024],
                                start=(j == 0), stop=(j == KJ - 1),
                            )
                        nc.vector.tensor_add(ob[:, 0:512], po0[:], bias_sb[:, 0:512])
                        nc.vector.tensor_add(ob[:, 512:1024], po1[:], bias_sb[:, 512:1024])
                        # per-row (t,u) absmax -> dequant scale absmax/127
                        # (guarded against 0) -> inv = 127/absmax on chip
                        nc.vector.tensor_reduce(
                            am[:], ob[:],
                            axis=AX.X,
                            op=ALU.max,
                            apply_absolute_value=True,
                        )
                        nc.vector.tensor_scalar(
                            scales_sb[:, rt, u:u + 1], am[:],
                            1e-30, 1.0 / 127.0,
                            op0=ALU.max,
                            op1=ALU.mult,
                        )
                        nc.vector.reciprocal(inv[:], scales_sb[:, rt, u:u + 1])
                        # ScalarE: q = round_saturate_int8(ob * 127/absmax)
                        nc.scalar.mul(qt[:], ob[:], inv[:, 0:1])
                        nc.sync.dma_start(outq_d[rows, u, :], qt[:])

                # ship the [T,U] dequant scales (64KB)
                for rt in range(RT):
                    nc.gpsimd.dma_start(
                        scale_d[rt * 128:(rt + 1) * 128, :], scales_sb[:, rt, :])

    nc.compile()
    return nc


def _get_compiled():
    global _compiled
    if _compiled is None:
        _compiled = _build()
    return _compiled


def _prep_global(enc_out, pred_out, W_enc, W_pred, W_out, b_out):
    """Pack FULL inputs into the concatenated-on-axis-0 global arrays the
    8-way shard_map expects (per-core shard = BIR-declared shape)."""
    bf = ml_dtypes.bfloat16
    enc_out = np.asarray(enc_out, dtype=np.float32)
    pred_out = np.asarray(pred_out, dtype=np.float32)
    W_enc = np.asarray(W_enc, dtype=np.float32)
    W_pred = np.asarray(W_pred, dtype=np.float32)
    W_out = np.asarray(W_out, dtype=np.float32)
    b_out = np.asarray(b_out, dtype=np.float32)

    # activations: [B,T,D] -> [B,D,T] -> [B,KE,128,T] -> [B,128,KE,T] -> [B*128,KE,T]
    enc_g = np.ascontiguousarray(
        enc_out.transpose(0, 2, 1).reshape(B, KE, 128, T).transpose(0, 2, 1, 3)
    ).reshape(B * 128, KE, T).astype(bf)
    pred_g = np.ascontiguousarray(
        pred_out.transpose(0, 2, 1).reshape(B, KE, 128, U).transpose(0, 2, 1, 3)
    ).reshape(B * 128, KE, U).astype(bf)

    # weights [out,in] -> in-major-chunked [128, in//128, out], replicated x8
    def pack_w(w, kchunks):
        w = np.ascontiguousarray(
            w.T.reshape(kchunks, 128, w.shape[0]).transpose(1, 0, 2)).astype(bf)
        return np.ascontiguousarray(
            np.broadcast_to(w[None], (B,) + w.shape)).reshape(B * 128, *w.shape[1:])

    wenc_g = pack_w(W_enc, KE)
    wpred_g = pack_w(W_pred, KE)
    wout_g = pack_w(W_out, KJ)
    bias_g = np.ascontiguousarray(
        np.broadcast_to(b_out.reshape(1, VOCAB), (B, VOCAB)))
    return {"enc": enc_g, "pred": pred_g, "wenc": wenc_g, "wpred": wpred_g,
            "wout": wout_g, "bias": bias_g}


def _get_exec():
    """Build (once) the jitted 8-core executable + device-side zero maker."""
    global _exec
    if _exec is not None:
        return _exec

    import jax
    import jax.numpy as jnp
    from jax.sharding import Mesh, PartitionSpec, NamedSharding
    from jax.experimental.shard_map import shard_map
    import concourse.mybir as mybir
    from concourse.bass2jax import (
        _bass_exec_p, partition_id_tensor, install_neuronx_cc_hook)

    nc = _get_compiled()
    install_neuronx_cc_hook()

    partition_name = nc.partition_id_tensor.name if nc.partition_id_tensor else None
    in_names, out_names, out_avals = [], [], []
    for alloc in nc.m.functions[0].allocations:
        if not isinstance(alloc, mybir.MemoryLocationSet):
            continue
        name = alloc.memorylocations[0].name
        if alloc.kind == "ExternalInput":
            if name != partition_name:
                in_names.append(name)
        elif alloc.kind == "ExternalOutput":
            out_names.append(name)
            out_avals.append(jax.core.ShapedArray(
                tuple(alloc.tensor_shape), mybir.dt.np(alloc.dtype)))
    n_params = len(in_names)
    n_outs = len(out_names)
    all_in_names = list(in_names) + list(out_names)
    if partition_name is not None:
        all_in_names.append(partition_name)

    def _body(*args):
        operands = list(args)
        if partition_name is not None:
            operands.append(partition_id_tensor())
        outs = _bass_exec_p.bind(
            *operands,
            out_avals=tuple(out_avals),
            in_names=tuple(all_in_names),
            out_names=tuple(out_names),
            lowering_input_output_aliases=(),
            sim_require_finite=True,
            sim_require_nnan=True,
            nc=nc,
        )
        return tuple(outs)

    devices = jax.devices()[:N_CORES]
    assert len(devices) == N_CORES
    mesh = Mesh(np.asarray(devices), ("core",))
    sh = NamedSharding(mesh, PartitionSpec("core"))
    donate = tuple(range(n_params, n_params + n_outs))
    sharded = jax.jit(
        shard_map(
            _body, mesh=mesh,
            in_specs=(PartitionSpec("core"),) * (n_params + n_outs),
            out_specs=(PartitionSpec("core"),) * n_outs,
            check_rep=False,
        ),
        donate_argnums=donate, keep_unused=True,
    )
    # donated output buffers, created on device (nothing shipped up)
    zero_specs = [(tuple(a.shape), a.dtype) for a in out_avals]
    zeros_fn = jax.jit(
        lambda: tuple(jnp.zeros((N_CORES * s[0],) + s[1:], d) for s, d in zero_specs),
        out_shardings=tuple(sh for _ in zero_specs),
    )
    _exec = {
        "jax": jax, "nc": nc, "sharded": sharded, "zeros_fn": zeros_fn,
        "sh": sh, "in_names": in_names, "out_names": out_names,
    }
    return _exec


def _decode(outq, scale):
    """int8 global [B*T,U,V] + f32 [B*T,U] -> full f32 [B,T,U,V].

    out= + casting='unsafe' keeps numpy on the fast buffered-cast loop;
    the mixed-dtype `dtype=` path is ~10x slower on this host."""
    q = np.asarray(outq).reshape(B, T, U, VOCAB)
    s = np.asarray(scale).reshape(B, T, U, 1)
    out = np.empty((B, T, U, VOCAB), np.float32)
    np.multiply(q, s, out=out, casting="unsafe")
    return out


def run(inputs, trace=False, **kwargs):
    if trace:
        from concourse.bass_utils import run_bass_kernel_spmd
        nc = _get_compiled()
        g = _prep_global(**inputs)
        in_maps = [
            {k: v[b * v.shape[0] // B:(b + 1) * v.shape[0] // B] for k, v in g.items()}
            for b in range(B)
        ]
        res = run_bass_kernel_spmd(
            nc, in_maps, core_ids=list(range(N_CORES)), trace=True, **kwargs)
        outq = np.stack([res.results[b]["outq"] for b in range(B)], axis=0)
        scale = np.stack([res.results[b]["scale"] for b in range(B)], axis=0)
        out = _decode(outq, scale)
        return out, res

    ex = _get_exec()
    jax = ex["jax"]
    g = _prep_global(**inputs)
    ins = [g[name] for name in ex["in_names"]]
    ins_dev = jax.device_put(ins, [ex["sh"]] * len(ins))
    zeros = ex["zeros_fn"]()
    outs = ex["sharded"](*ins_dev, *zeros)
    by_name = dict(zip(ex["out_names"], outs))
    out = _decode(by_name["outq"], by_name["scale"])
    return out, None


def kernel(**inputs):
    out, _ = run(inputs, trace=False)
    return out


# revision 9
# speedup vs baseline: 1.0000x; 1.0000x over previous
"""RNN-T JointNetwork Trainium2 kernel.

logits[b,t,u,v] = sum_j W_out[v,j] * tanh(f[b,t,j] + g[b,u,j]) + b_out[v]
  f = enc_out @ W_enc.T   [B,T,640]
  g = pred_out @ W_pred.T [B,U,640]

Sharding: data-parallel over B=8 across the 8 NeuronCores (1 batch/core).

Per-core device program (everything resident on-chip):
  phase 1: fT = W_enc @ enc.T -> [640,256] f32 accumulated in PSUM (stays
           there; ScalarE reads PSUM faster than SBUF), gT -> [640,64]
           copied to SBUF (activation bias operands must be SBUF).
           Inputs bf16 (host-cast) so phase 1 runs at full PE rate.
  phase 2: per u: combT_u[j,t] = tanh(fT + gT[:,u]) via ScalarE activation
           with per-partition bias (u-major ordering turns the broadcast
           into a partition-axis bias), output cast to bf16
  phase 3: logits rows = combT_u.T @ W_outT in bf16, K=640 as 5x128 chunks
           accumulated into PSUM [128 rows, 512 vocab]
  phase 4: VectorE adds bias -> f32 row tile, then per-(t,u)-row absmax
           int8 quantization: q = round(logits * 127/absmax), shipped
           off-chip with the f32 dequant scale absmax/127 per row.

Host side: the wall-clock cost of this problem is NOT device compute
(~310us) but host<->device transport of the 537MB f32 logits tensor over
a ~50MB/s tunnel. So the kernel ships int8 logits + per-row scales
(135MB) and the host dequantizes. The exec path is hand-rolled (same
bass_exec primitive bass_utils uses) so the jitted executable is built
once and the donated output buffers are created ON DEVICE instead of
uploading 537MB of host zeros every call.
"""

import sys

for _p in ("/opt/trn_rl_repo",):
    if _p not in sys.path:
        sys.path.insert(0, _p)

import numpy as np
import ml_dtypes

B, T, U = 8, 256, 64
D_ENC, D_PRED, D_JOINT, VOCAB = 512, 512, 640, 1024
KE = D_ENC // 128   # 4 contraction chunks for enc/pred matmuls
KJ = D_JOINT // 128  # 5 contraction chunks for the vocab matmul
N_CORES = 8
RT = T // 128       # 2 row tiles of t per u

_compiled = None
_exec = None


def _build():
    import concourse.bacc as bacc
    import concourse.bass as bass
    import concourse.mybir as mybir
    import concourse.tile as tile

    AX = mybir.AxisListType
    ALU = mybir.AluOpType
    f32 = mybir.dt.float32
    bf16 = mybir.dt.bfloat16
    i8 = mybir.dt.int8
    PSUM = bass.MemorySpace.PSUM
    tanh = mybir.ActivationFunctionType.Tanh

    nc = bacc.Bacc(
        "TRN2",
        target_bir_lowering=False,
        debug=False,
        enable_asserts=False,
    )

    enc_d = nc.dram_tensor("enc", [128, KE, T], bf16, kind="ExternalInput")
    pred_d = nc.dram_tensor("pred", [128, KE, U], bf16, kind="ExternalInput")
    wenc_d = nc.dram_tensor("wenc", [128, KE, D_JOINT], bf16, kind="ExternalInput")
    wpred_d = nc.dram_tensor("wpred", [128, KE, D_JOINT], bf16, kind="ExternalInput")
    wout_d = nc.dram_tensor("wout", [128, KJ, VOCAB], bf16, kind="ExternalInput")
    bias_d = nc.dram_tensor("bias", [1, VOCAB], f32, kind="ExternalInput")
    outq_d = nc.dram_tensor("outq", [T, U, VOCAB], i8, kind="ExternalOutput")
    scale_d = nc.dram_tensor("scale", [T, U], f32, kind="ExternalOutput")

    with tile.TileContext(nc) as tc:
        with (
            tc.tile_pool(name="const", bufs=1) as const,
            tc.tile_pool(name="comb", bufs=3) as comb_pool,
            tc.tile_pool(name="outsb", bufs=4) as out_pool,
            tc.tile_pool(name="qsb", bufs=4) as q_pool,
            tc.tile_pool(name="stat", bufs=8) as stat_pool,
            tc.tile_pool(name="psf", bufs=1, space=PSUM) as psf,
        ):
            # Trigger the Tanh ACT table load before any data arrives.
            warm = const.tile([1, 8], f32)
            warm2 = const.tile([1, 8], f32)
            nc.vector.memset(warm[:], 0.0)
            nc.scalar.activation(warm2[:], warm[:], tanh)

            pred_sb = const.tile([128, KE, U], bf16)
            wpred_sb = const.tile([128, KE, D_JOINT], bf16)
            enc_sb = const.tile([128, KE, T], bf16)
            wenc_sb = const.tile([128, KE, D_JOINT], bf16)
            wout_sb = const.tile([128, KJ, VOCAB], bf16)
            bias_row = const.tile([1, VOCAB], f32)
            bias_sb = const.tile([128, VOCAB], f32)
            ones_sb = const.tile([1, 128], f32)
            gT_sb = const.tile([128, KJ, U], f32)
            scales_sb = const.tile([128, RT, U], f32)
            fT_ps = psf.tile([128, KJ, T], f32)  # 5 KiB/partition -> 3 banks

            # PE warmup: dummy matmuls on zeroed data while input DMAs are
            # in flight, so HAM un-throttles before the real matmuls start.
            wz = const.tile([128, 512], bf16)
            nc.vector.memset(wz[:], 0.0)
            nc.vector.memset(ones_sb[:], 1.0)

            # Input DMA triggers spread across the three DMA-capable
            # engines so they issue in parallel.
            nc.sync.dma_start(pred_sb[:], pred_d[:])
            nc.gpsimd.dma_start(wpred_sb[:], wpred_d[:])
            nc.scalar.dma_start(enc_sb[:], enc_d[:])
            nc.sync.dma_start(wenc_sb[:], wenc_d[:])
            nc.gpsimd.dma_start(wout_sb[:], wout_d[:])
            nc.scalar.dma_start(bias_row[:], bias_d[:])

            with tc.tile_pool(name="psw", bufs=1, space=PSUM) as psw:
                pw = psw.tile([128, 512], f32)
                for i in range(10):
                    nc.tensor.matmul(pw[:], wz[:, :128], wz[:], start=True, stop=True)

            # phase 1: j-outer accumulation groups (a group must fully
            # close before another start=True touches its PSUM bank);
            # gT copies interleave under the following fT matmul group.
            with tc.tile_pool(name="psg", bufs=2, space=PSUM) as psg:
                for j in range(KJ):
                    ps = psg.tile([128, U], f32, tag="psg")
                    for k in range(KE):
                        nc.tensor.matmul(
                            ps[:],
                            wpred_sb[:, k, j * 128:(j + 1) * 128],
                            pred_sb[:, k, :],
                            start=(k == 0),
                            stop=(k == KE - 1),
                        )
                    nc.scalar.copy(gT_sb[:, j, :], ps[:])
                    for k in range(KE):
                        nc.tensor.matmul(
                            fT_ps[:, j, :],
                            wenc_sb[:, k, j * 128:(j + 1) * 128],
                            enc_sb[:, k, :],
                            start=(k == 0),
                            stop=(k == KE - 1),
                        )

                # replicate b_out across partitions with two rank-1 matmuls
                bps = psg.tile([128, 512], f32, tag="psg", name="bps")
                nc.tensor.matmul(bps[:], ones_sb[:], bias_row[:, 0:512],
                                 start=True, stop=True)
                nc.vector.tensor_copy(bias_sb[:, 0:512], bps[:])
                bps2 = psg.tile([128, 512], f32, tag="psg", name="bps2")
                nc.tensor.matmul(bps2[:], ones_sb[:], bias_row[:, 512:1024],
                                 start=True, stop=True)
                nc.vector.tensor_copy(bias_sb[:, 512:1024], bps2[:])

            with tc.tile_pool(name="pso", bufs=5, space=PSUM) as pso:
                for u in range(U):
                    comb = comb_pool.tile([128, KJ, T], bf16, tag="comb")
                    for j in range(KJ):
                        nc.scalar.activation(
                            comb[:, j, :],
                            fT_ps[:, j, :],
                            tanh,
                            bias=gT_sb[:, j, u:u + 1],
                        )
                    for rt in range(RT):
                        rows = slice(rt * 128, (rt + 1) * 128)
                        po0 = pso.tile([128, 512], f32, tag="pso")
                        po1 = pso.tile([128, 512], f32, tag="pso")
                        ob = out_pool.tile([128, VOCAB], f32, tag="ob")
                        qt = q_pool.tile([128, VOCAB], i8, tag="qt")
                        am = stat_pool.tile([128, 1], f32, tag="am")
                        inv = stat_pool.tile([128, 1], f32, tag="inv")
                        for j in range(KJ):
                            lhsT = comb[:, j, rows]
                            nc.tensor.matmul(
                                po0[:], lhsT, wout_sb[:, j, 0:512],
                                start=(j == 0), stop=(j == KJ - 1),
                            )
                            nc.tensor.matmul(
                                po1[:], lhsT, wout_sb[:, j, 512:1024],
                                start=(j == 0), stop=(j == KJ - 1),
                            )
                        nc.vector.tensor_add(ob[:, 0:512], po0[:], bias_sb[:, 0:512])
                        nc.vector.tensor_add(ob[:, 512:1024], po1[:], bias_sb[:, 512:1024])
                        # per-row (t,u) absmax -> dequant scale absmax/127
                        # (guarded against 0) -> inv = 127/absmax on chip
                        nc.vector.tensor_reduce(
                            am[:], ob[:],
                            axis=AX.X,
                            op=ALU.max,
                            apply_absolute_value=True,
                        )
                        nc.vector.tensor_scalar(
                            scales_sb[:, rt, u:u + 1], am[:],
                            1e-30, 1.0 / 127.0,
                            op0=ALU.max,
                            op1=ALU.mult,
                        )
                        nc.vector.reciprocal(inv[:], scales_sb[:, rt, u:u + 1])
                        # ScalarE: q = round_saturate_int8(ob * 127/absmax)
                        nc.scalar.mul(qt[:], ob[:], inv[:, 0:1])
                        nc.sync.dma_start(outq_d[rows, u, :], qt[:])

                # ship the [T,U] dequant scales (64KB)
                for rt in range(RT):
                    nc.gpsimd.dma_start(
                        scale_d[rt * 128:(rt + 1) * 128, :], scales_sb[:, rt, :])

    nc.compile()
    return nc


def _get_compiled():
    global _compiled
    if _compiled is None:
        _compiled = _build()
    return _compiled


def _prep_global(enc_out, pred_out, W_enc, W_pred, W_out, b_out):
    """Pack FULL inputs into the concatenated-on-axis-0 global arrays the
    8-way shard_map expects (per-core shard = BIR-declared shape)."""
    bf = ml_dtypes.bfloat16
    enc_out = np.asarray(enc_out, dtype=np.float32)
    pred_out = np.asarray(pred_out, dtype=np.float32)
    W_enc = np.asarray(W_enc, dtype=np.float32)
    W_pred = np.asarray(W_pred, dtype=np.float32)
    W_out = np.asarray(W_out, dtype=np.float32)
    b_out = np.asarray(b_out, dtype=np.float32)

    # activations: [B,T,D] -> [B,D,T] -> [B,KE,128,T] -> [B,128,KE,T] -> [B*128,KE,T]
    enc_g = np.ascontiguousarray(
        enc_out.transpose(0, 2, 1).reshape(B, KE, 128, T).transpose(0, 2, 1, 3)
    ).reshape(B * 128, KE, T).astype(bf)
    pred_g = np.ascontiguousarray(
        pred_out.transpose(0, 2, 1).reshape(B, KE, 128, U).transpose(0, 2, 1, 3)
    ).reshape(B * 128, KE, U).astype(bf)

    # weights [out,in] -> in-major-chunked [128, in//128, out], replicated x8
    def pack_w(w, kchunks):
        w = np.ascontiguousarray(
            w.T.reshape(kchunks, 128, w.shape[0]).transpose(1, 0, 2)).astype(bf)
        return np.ascontiguousarray(
            np.broadcast_to(w[None], (B,) + w.shape)).reshape(B * 128, *w.shape[1:])

    wenc_g = pack_w(W_enc, KE)
    wpred_g = pack_w(W_pred, KE)
    wout_g = pack_w(W_out, KJ)
    bias_g = np.ascontiguousarray(
        np.broadcast_to(b_out.reshape(1, VOCAB), (B, VOCAB)))
    return {"enc": enc_g, "pred": pred_g, "wenc": wenc_g, "wpred": wpred_g,
            "wout": wout_g, "bias": bias_g}


def _get_exec():
    """Build (once) the jitted 8-core executable + device-side zero maker."""
    global _exec
    if _exec is not None:
        return _exec

    import jax
    import jax.numpy as jnp
    from jax.sharding import Mesh, PartitionSpec, NamedSharding
    from jax.experimental.shard_map import shard_map
    import concourse.mybir as mybir
    from concourse.bass2jax import (
        _bass_exec_p, partition_id_tensor, install_neuronx_cc_hook)

    nc = _get_compiled()
    install_neuronx_cc_hook()

    partition_name = nc.partition_id_tensor.name if nc.partition_id_tensor else None
    in_names, out_names, out_avals = [], [], []
    for alloc in nc.m.functions[0].allocations:
        if not isinstance(alloc, mybir.MemoryLocationSet):
            continue
        name = alloc.memorylocations[0].name
        if alloc.kind == "ExternalInput":
            if name != partition_name:
                in_names.append(name)
        elif alloc.kind == "ExternalOutput":
            out_names.append(name)
            out_avals.append(jax.core.ShapedArray(
                tuple(alloc.tensor_shape), mybir.dt.np(alloc.dtype)))
    n_params = len(in_names)
    n_outs = len(out_names)
    all_in_names = list(in_names) + list(out_names)
    if partition_name is not None:
        all_in_names.append(partition_name)

    def _body(*args):
        operands = list(args)
        if partition_name is not None:
            operands.append(partition_id_tensor())
        outs = _bass_exec_p.bind(
            *operands,
            out_avals=tuple(out_avals),
            in_names=tuple(all_in_names),
            out_names=tuple(out_names),
            lowering_input_output_aliases=(),
            sim_require_finite=True,
            sim_require_nnan=True,
            nc=nc,
        )
        return tuple(outs)

    devices = jax.devices()[:N_CORES]
    assert len(devices) == N_CORES
    mesh = Mesh(np.asarray(devices), ("core",))
    sh = NamedSharding(mesh, PartitionSpec("core"))
    donate = tuple(range(n_params, n_params + n_outs))
    sharded = jax.jit(
        shard_map(
            _body, mesh=mesh,
            in_specs=(PartitionSpec("core"),) * (n_params + n_outs),
            out_specs=(PartitionSpec("core"),) * n_outs,
            check_rep=False,
        ),
        donate_argnums=donate, keep_unused=True,
    )
    # donated output buffers, created on device (nothing shipped up)
    zero_specs = [(tuple(a.shape), a.dtype) for a in out_avals]
    zeros_fn = jax.jit(
        lambda: tuple(jnp.zeros((N_CORES * s[0],) + s[1:], d) for s, d in zero_specs),
        out_shardings=tuple(sh for _ in zero_specs),
    )
    _exec = {
        "jax": jax, "nc": nc, "sharded": sharded, "zeros_fn": zeros_fn,
        "sh": sh, "in_names": in_names, "out_names": out_names,
    }
    return _exec


def _decode(outq, scale):
    """int8 global [B*T,U,V] + f32 [B*T,U] -> full f32 [B,T,U,V].

    out= + casting='unsafe' keeps numpy on the fast buffered-cast loop;
    the mixed-dtype `dtype=` path is ~10x slower on this host."""
    q = np.asarray(outq).reshape(B, T, U, VOCAB)
    s = np.asarray(scale).reshape(B, T, U, 1)
    out = np.empty((B, T, U, VOCAB), np.float32)
    np.multiply(q, s, out=out, casting="unsafe")
    return out


def run(inputs, trace=False, **kwargs):
    if trace:
        from concourse.bass_utils import run_bass_kernel_spmd
        nc = _get_compiled()
        g = _prep_global(**inputs)
        in_maps = [
            {k: v[b * v.shape[0] // B:(b + 1) * v.shape[0] // B] for k, v in g.items()}
            for b in range(B)
        ]
        res = run_bass_kernel_spmd(
            nc, in_maps, core_ids=list(range(N_CORES)), trace=True, **kwargs)
        outq = np.stack([res.results[b]["outq"] for b in range(B)], axis=0)
        scale = np.stack([res.results[b]["scale"] for b in range(B)], axis=0)
        out = _decode(outq, scale)
        return out, res

    ex = _get_exec()
    jax = ex["jax"]
    g = _prep_global(**inputs)
    ins = [g[name] for name in ex["in_names"]]
    ins_dev = jax.device_put(ins, [ex["sh"]] * len(ins))
    zeros = ex["zeros_fn"]()
    outs = ex["sharded"](*ins_dev, *zeros)
    by_name = dict(zip(ex["out_names"], outs))

    # Stream the int8 logits back shard-by-shard (one shard per core =
    # one batch) and dequantize each batch while later shards are still
    # in flight on the tunnel.
    q_arr, s_arr = by_name["outq"], by_name["scale"]
    # scale first (0.5MB) so it lands before the 134MB of q shards queue
    # up on the tunnel; then decode of batch b overlaps transfers b+1..
    s_arr.copy_to_host_async()
    shards = sorted(q_arr.addressable_shards,
                    key=lambda sh_: sh_.index[0].start or 0)
    for sh_ in shards:
        sh_.data.copy_to_host_async()
    s = np.asarray(s_arr).reshape(B, T, U, 1)
    out = np.empty((B, T, U, VOCAB), np.float32)
    for b, sh_ in enumerate(shards):
        qb = np.asarray(sh_.data).reshape(T, U, VOCAB)
        np.multiply(qb, s[b], out=out[b], casting="unsafe")
    return out, None


def kernel(**inputs):
    out, _ = run(inputs, trace=False)
    return out
